# revision 1
# baseline (speedup 1.0000x reference)
"""Trainium2 Bass kernel for the CrossFunctionsLoss problem.

Computes, for S in {SU, SP, SM} (each [N,N]) and FP, FM, B ([D,N]):
    Omega_u = 0.5 * FP^T FM ; Omega_p = 0.5 * FP^T FP ; Omega_m = 0.5 * FM^T FM
    loss = sum(-SU*Om_u + log1p(Om_u)) + sum(-SP*Om_p + log(1+Om_p+eps))
         + sum(-SM*Om_m + log(1+Om_m+eps))
         + ||FP-B||_F + ||FM-B||_F + sum(rowsum(FP)^2) + sum(rowsum(FM)^2)

Sharding: data-parallel over the N (item) axis. Each of the 8 cores gets a
512-row block of SU/SP/SM and the matching 512 columns of FP/FM/B, plus full
FP/FM for the GEMM moving operand. Each core emits per-partition partial sums;
the host does the final (tiny) combine in float64.

Per core, per pairing: 4 row-strips of 128; per strip, two [128,2048] PSUM
tiles each filled by 4 float32r matmuls, then drained by one ScalarE
Ln(0.5*g+1) pass with accum_out (the log term) and one VectorE
tensor_tensor_reduce (mult, scale=-0.5, reduce add) against the streamed S
strip (the -S*Omega term).
"""

import sys

if "/opt/trn_rl_repo" not in sys.path:
    sys.path.insert(0, "/opt/trn_rl_repo")

import numpy as np

import concourse.bass as bass
import concourse.tile as tile
from concourse import bacc, mybir
from concourse.bass_utils import run_bass_kernel_spmd

D = 128
N = 4096
N_CORES = 8
NL = N // N_CORES  # 512 rows of Omega per core
EPS = 1e-08

F32 = mybir.dt.float32
F32R = mybir.dt.float32r
ALU = mybir.AluOpType
ACTF = mybir.ActivationFunctionType

N_STRIPS = NL // 128          # 4 row strips per core
N_HALF = 2                    # two 2048-column halves per strip
HALF_W = N // N_HALF          # 2048
N_MAIN_COLS = 3 * N_STRIPS * N_HALF * 2   # 48 accumulator columns (log + dot)


def build_program(repeat=1, dyn_repeat=None):
    # Bacc (not plain Bass): its compile() runs generate_event_semaphores,
    # which splits multi-wait instructions into EventSemaphore helpers (TRN2
    # allows at most one sync wait per instruction).
    #
    # dyn_repeat: if set, the streaming body is wrapped in a tc.For_i loop
    # that re-reads the same DRAM inputs dyn_repeat times (timing only).
    nc = bacc.Bacc("TRN2", target_bir_lowering=False, debug=False)

    s_u = nc.dram_tensor("s_u", [NL, N], F32, kind="ExternalInput").ap()
    s_p = nc.dram_tensor("s_p", [NL, N], F32, kind="ExternalInput").ap()
    s_m = nc.dram_tensor("s_m", [NL, N], F32, kind="ExternalInput").ap()
    fp = nc.dram_tensor("fp", [D, N], F32, kind="ExternalInput").ap()
    fm = nc.dram_tensor("fm", [D, N], F32, kind="ExternalInput").ap()
    fp_loc = nc.dram_tensor("fp_loc", [D, NL], F32, kind="ExternalInput").ap()
    fm_loc = nc.dram_tensor("fm_loc", [D, NL], F32, kind="ExternalInput").ap()
    b_loc = nc.dram_tensor("b_loc", [D, NL], F32, kind="ExternalInput").ap()
    out = nc.dram_tensor("out", [D, 8], F32, kind="ExternalOutput").ap()

    with tile.TileContext(nc) as tc:
        with (
            tc.tile_pool(name="consts", bufs=1) as consts,
            tc.tile_pool(name="sstrip", bufs=6) as spool,
            tc.tile_pool(name="psum", bufs=2, space="PSUM") as pp,
            tc.tile_pool(name="scratch", bufs=1) as scratch,
        ):
            # Small per-core inputs first (they unblock the lhsT casts).
            fpl_sb = consts.tile([D, NL], F32, tag="fpl")
            nc.sync.dma_start(fpl_sb[:], fp_loc[:])
            fml_sb = consts.tile([D, NL], F32, tag="fml")
            nc.sync.dma_start(fml_sb[:], fm_loc[:])
            b_sb = consts.tile([D, NL], F32, tag="b")
            nc.sync.dma_start(b_sb[:], b_loc[:])
            # fp in halves so the first cast chunks start sooner.
            fp_sb = consts.tile([D, N], F32, tag="fp")
            nc.sync.dma_start(fp_sb[:, :HALF_W], fp[:, :HALF_W])
            nc.sync.dma_start(fp_sb[:, HALF_W:], fp[:, HALF_W:])
            # fm is not needed until pairing M (~1/3 in); loaded later.
            fm_sb = consts.tile([D, N], F32, tag="fm")

            # float32r-rounded copies for the matmul operands (the BIR
            # verifier requires fp32r matmul inputs to be produced by a
            # rounding instruction). GpSimd is otherwise idle; chunked so
            # the first matmuls are unblocked as early as possible.
            fpl_r = consts.tile([D, NL], F32R, tag="fpl_r")
            nc.gpsimd.tensor_copy(fpl_r[:], fpl_sb[:])
            fml_r = consts.tile([D, NL], F32R, tag="fml_r")
            nc.vector.tensor_copy(fml_r[:], fml_sb[:])
            fp_r = consts.tile([D, N], F32R, tag="fp_r")
            nc.gpsimd.tensor_copy(fp_r[:, :HALF_W], fp_sb[:, :HALF_W])
            nc.gpsimd.tensor_copy(fp_r[:, HALF_W:], fp_sb[:, HALF_W:])
            fm_r = consts.tile([D, N], F32R, tag="fm_r")

            acc = consts.tile([D, 64], F32, tag="acc")
            out_sb = consts.tile([D, 8], F32, tag="out")
            nc.vector.memset(out_sb[:], 0.0)

            act_scr = scratch.tile([D, HALF_W], F32, tag="act_scr")
            dve_scr = scratch.tile([D, HALF_W], F32, tag="dve_scr")
            bqc_scr = scratch.tile([D, NL], F32, tag="bqc_scr")

            col = 0
            # (stationary local chunk, moving full tensor, S row-block)
            pairings = [
                (fpl_r, fp_r, s_p),   # Omega_p: needs only FP -> first
                (fml_r, fm_r, s_m),   # Omega_m
                (fpl_r, fm_r, s_u),   # Omega_u
            ]
            def load_fm():
                # Load + cast FM just before pairing M needs it, so the
                # first pairing's S strips get the DMA device early.
                nc.sync.dma_start(fm_sb[:, :HALF_W], fm[:, :HALF_W])
                nc.sync.dma_start(fm_sb[:, HALF_W:], fm[:, HALF_W:])
                nc.gpsimd.tensor_copy(fm_r[:, :HALF_W], fm_sb[:, :HALF_W])
                nc.gpsimd.tensor_copy(fm_r[:, HALF_W:], fm_sb[:, HALF_W:])

            def stream_body(with_fm_load):
                col = 0
                for pi, (loc_sb, full_sb, s_dram) in enumerate(pairings):
                    if pi == 1 and with_fm_load:
                        load_fm()
                    for m in range(N_STRIPS):
                        lhsT = loc_sb[:, m * 128 : (m + 1) * 128]
                        for h in range(N_HALF):
                            s_half = spool.tile([128, HALF_W], F32, tag="s_half")
                            nc.sync.dma_start(
                                s_half[:],
                                s_dram[
                                    m * 128 : (m + 1) * 128,
                                    h * HALF_W : (h + 1) * HALF_W,
                                ],
                            )
                            ps = pp.tile([128, HALF_W], F32, tag="ps")
                            for q in range(4):
                                j = h * 4 + q
                                nc.tensor.matmul(
                                    ps[:, q * 512 : (q + 1) * 512],
                                    lhsT,
                                    full_sb[:, j * 512 : (j + 1) * 512],
                                    start=True,
                                    stop=True,
                                )
                            # log(1 + 0.5*g) summed along free dim -> acc col
                            nc.scalar.activation(
                                act_scr[:],
                                ps[:],
                                ACTF.Ln,
                                bias=1.0,
                                scale=0.5,
                                accum_out=acc[:, col : col + 1],
                            )
                            col += 1
                            # sum((g * -0.5) * s) -> acc col
                            nc.vector.scalar_tensor_tensor(
                                out=dve_scr[:],
                                in0=ps[:],
                                scalar=-0.5,
                                in1=s_half[:],
                                op0=ALU.mult,
                                op1=ALU.mult,
                                accum_out=acc[:, col : col + 1],
                            )
                            col += 1
                assert col == N_MAIN_COLS

            if dyn_repeat is not None:
                load_fm()
                with tc.For_i(0, dyn_repeat, 1):
                    stream_body(with_fm_load=False)
            else:
                for rep in range(repeat):
                    stream_body(with_fm_load=(rep == 0))

            # BQC partials: sum((F_loc - B_loc)^2) per partition. On GpSimd
            # (otherwise idle) to keep DVE free for the main dot passes.
            nc.gpsimd.tensor_tensor(
                out=bqc_scr[:], in0=fpl_sb[:], in1=b_sb[:], op=ALU.subtract
            )
            nc.vector.scalar_tensor_tensor(
                out=bqc_scr[:],
                in0=bqc_scr[:],
                scalar=1.0,
                in1=bqc_scr[:],
                op0=ALU.mult,
                op1=ALU.mult,
                accum_out=acc[:, N_MAIN_COLS : N_MAIN_COLS + 1],
            )
            bqc_scr2 = scratch.tile([D, NL], F32, tag="bqc_scr2")
            nc.gpsimd.tensor_tensor(
                out=bqc_scr2[:], in0=fml_sb[:], in1=b_sb[:], op=ALU.subtract
            )
            nc.vector.scalar_tensor_tensor(
                out=bqc_scr2[:],
                in0=bqc_scr2[:],
                scalar=1.0,
                in1=bqc_scr2[:],
                op0=ALU.mult,
                op1=ALU.mult,
                accum_out=acc[:, N_MAIN_COLS + 1 : N_MAIN_COLS + 2],
            )

            # FDC rowsums of the full FP/FM (identical on every core; host
            # uses core 0's). fp on ScalarE (Copy + accum_out), fm on DVE.
            nc.scalar.activation(
                act_scr[:],
                fp_sb[:, :HALF_W],
                ACTF.Copy,
                bias=0.0,
                scale=1.0,
                accum_out=acc[:, N_MAIN_COLS + 2 : N_MAIN_COLS + 3],
            )
            nc.scalar.activation(
                act_scr[:],
                fp_sb[:, HALF_W:],
                ACTF.Copy,
                bias=0.0,
                scale=1.0,
                accum_out=acc[:, N_MAIN_COLS + 3 : N_MAIN_COLS + 4],
            )
            nc.vector.tensor_reduce(
                out=acc[:, N_MAIN_COLS + 4 : N_MAIN_COLS + 5],
                in_=fm_sb[:],
                axis=mybir.AxisListType.X,
                op=ALU.add,
            )

            # Fold the 48 main columns into out col 0.
            nc.vector.tensor_reduce(
                out=out_sb[:, 0:1],
                in_=acc[:, 0:N_MAIN_COLS],
                axis=mybir.AxisListType.X,
                op=ALU.add,
            )
            # bqc_p, bqc_m -> cols 1,2
            nc.vector.tensor_copy(
                out_sb[:, 1:3], acc[:, N_MAIN_COLS : N_MAIN_COLS + 2]
            )
            # fp rowsum halves -> cols 3,4 ; fm rowsum -> col 5
            nc.vector.tensor_copy(
                out_sb[:, 3:6], acc[:, N_MAIN_COLS + 2 : N_MAIN_COLS + 5]
            )

            nc.sync.dma_start(out[:], out_sb[:])

    nc.compile()
    return nc


_NC_CACHE = None


def _get_program():
    global _NC_CACHE
    if _NC_CACHE is None:
        _NC_CACHE = build_program()
    return _NC_CACHE


def make_in_maps(SU, SP, SM, FP, FM, B):
    SU = np.ascontiguousarray(np.asarray(SU, np.float32).reshape(N, N))
    SP = np.ascontiguousarray(np.asarray(SP, np.float32).reshape(N, N))
    SM = np.ascontiguousarray(np.asarray(SM, np.float32).reshape(N, N))
    FP = np.ascontiguousarray(np.asarray(FP, np.float32))
    FM = np.ascontiguousarray(np.asarray(FM, np.float32))
    B = np.ascontiguousarray(np.asarray(B, np.float32))
    in_maps = []
    for k in range(N_CORES):
        sl = slice(k * NL, (k + 1) * NL)
        in_maps.append(
            {
                "s_u": np.ascontiguousarray(SU[sl]),
                "s_p": np.ascontiguousarray(SP[sl]),
                "s_m": np.ascontiguousarray(SM[sl]),
                "fp": FP,
                "fm": FM,
                "fp_loc": np.ascontiguousarray(FP[:, sl]),
                "fm_loc": np.ascontiguousarray(FM[:, sl]),
                "b_loc": np.ascontiguousarray(B[:, sl]),
            }
        )
    return in_maps


def combine_outs(outs):
    """outs: list of 8 [128, 8] float32 arrays -> scalar loss (float32)."""
    outs = [np.asarray(o, np.float64) for o in outs]
    main = sum(o[:, 0].sum() for o in outs)
    bqc = np.sqrt(sum(o[:, 1].sum() for o in outs)) + np.sqrt(
        sum(o[:, 2].sum() for o in outs)
    )
    rs_fp = outs[0][:, 3] + outs[0][:, 4]
    rs_fm = outs[0][:, 5]
    fdc = np.square(rs_fp).sum() + np.square(rs_fm).sum()
    return np.float32(main + bqc + fdc)


def kernel(SU, SP, SM, FP, FM, B):
    nc = _get_program()
    in_maps = make_in_maps(SU, SP, SM, FP, FM, B)
    res = run_bass_kernel_spmd(nc, in_maps, list(range(N_CORES)))
    return combine_outs([res.results[k]["out"] for k in range(N_CORES)])


if __name__ == "__main__":
    rng = np.random.default_rng(0)
    ins = {
        "SU": rng.random((N, N, 1), np.float32),
        "SP": rng.random((N, N, 1), np.float32),
        "SM": rng.random((N, N, 1), np.float32),
        "FP": rng.random((D, N), np.float32),
        "FM": rng.random((D, N), np.float32),
        "B": rng.random((D, N), np.float32),
    }
    got = kernel(**ins)
    print("kernel:", got)



# revision 2
# speedup vs baseline: 269.2570x; 269.2570x over previous
"""Trainium2 Bass kernel for the CrossFunctionsLoss problem.

Computes, for S in {SU, SP, SM} (each [N,N]) and FP, FM, B ([D,N]):
    Omega_u = 0.5 * FP^T FM ; Omega_p = 0.5 * FP^T FP ; Omega_m = 0.5 * FM^T FM
    loss = sum(-SU*Om_u + log1p(Om_u)) + sum(-SP*Om_p + log(1+Om_p+eps))
         + sum(-SM*Om_m + log(1+Om_m+eps))
         + ||FP-B||_F + ||FM-B||_F + sum(rowsum(FP)^2) + sum(rowsum(FM)^2)

Sharding: data-parallel over the N (item) axis. Each of the 8 cores gets a
512-row block of SU/SP/SM and the matching 512 columns of FP/FM/B, plus full
FP/FM for the GEMM moving operand. Each core emits per-partition partial sums;
the host does the final (tiny) combine in float64.

The kernel is DMA-bound (24 MB of S per core). The log term needs an explicit
Omega (PE matmul -> ScalarE Ln with accum_out). The -S*Omega term does NOT: it
is rewritten as a second GEMM on the otherwise idle PE,
    sum_ij S_ij * Omega_ij = 0.5 * sum_dj F2[d,j] * U[d,j],
    U[d,j] = sum_i F1[d,i] * S[i,j]   (lhsT = F1_loc^T chunks, rhs = S tiles),
leaving the DVE only a [128,1024] dot per column block (12 per iteration
instead of 24 full passes  — the DVE elementwise pass was measured to be the
bottleneck at ~2x the DMA floor). S tiles are declared float32r in DRAM
(bit-identical to f32) so they feed the matmul at full rate straight from DMA.

Per pairing: 4 column blocks of 1024; per block, 4 row strips of 128. Omega
strip tiles [128,1024] (2 PSUM banks, double-buffered) are drained by ScalarE
Ln(0.5*g+1) with accum_out; U accumulates across the 4 strips into a
[128,1024] PSUM tile (double-buffered), then one DVE scalar_tensor_tensor
against F2 produces the dot partial.
"""

import sys

if "/opt/trn_rl_repo" not in sys.path:
    sys.path.insert(0, "/opt/trn_rl_repo")

import numpy as np

import concourse.bass as bass
import concourse.tile as tile
from concourse import bacc, mybir
from concourse.bass_utils import run_bass_kernel_spmd

D = 128
N = 4096
N_CORES = 8
NL = N // N_CORES  # 512 rows of Omega per core
EPS = 1e-08

F32 = mybir.dt.float32
F32R = mybir.dt.float32r
ALU = mybir.AluOpType
ACTF = mybir.ActivationFunctionType

N_STRIPS = NL // 128          # 4 row strips per core
N_BLOCKS = 4                  # four 1024-column blocks
BLK_W = N // N_BLOCKS         # 1024
N_MAIN_COLS = 3 * (N_BLOCKS * N_STRIPS + N_BLOCKS)  # 48 log + 12 dot = 60


def build_program(repeat=1, dyn_repeat=None):
    # Bacc (not plain Bass): its compile() runs generate_event_semaphores,
    # which splits multi-wait instructions into EventSemaphore helpers (TRN2
    # allows at most one sync wait per instruction).
    #
    # dyn_repeat: if set, the streaming body is wrapped in a tc.For_i loop
    # that re-reads the same DRAM inputs dyn_repeat times (timing only).
    nc = bacc.Bacc("TRN2", target_bir_lowering=False, debug=False)

    # S tensors and F tensors feeding the PE are declared float32r: the bits
    # are plain f32, the tag lets DMA-loaded tiles be matmul operands at full
    # rate (verified on HW: micro_f32r.py).
    s_u = nc.dram_tensor("s_u", [NL, N], F32R, kind="ExternalInput").ap()
    s_p = nc.dram_tensor("s_p", [NL, N], F32R, kind="ExternalInput").ap()
    s_m = nc.dram_tensor("s_m", [NL, N], F32R, kind="ExternalInput").ap()
    fp = nc.dram_tensor("fp", [D, N], F32R, kind="ExternalInput").ap()
    fm = nc.dram_tensor("fm", [D, N], F32R, kind="ExternalInput").ap()
    fp_loc = nc.dram_tensor("fp_loc", [D, NL], F32R, kind="ExternalInput").ap()
    fm_loc = nc.dram_tensor("fm_loc", [D, NL], F32R, kind="ExternalInput").ap()
    # F_loc^T packed as [128, NL]: cols m*128+d hold F[d, off+m*128+p].
    fpl_t = nc.dram_tensor("fpl_t", [128, NL], F32R, kind="ExternalInput").ap()
    fml_t = nc.dram_tensor("fml_t", [128, NL], F32R, kind="ExternalInput").ap()
    # f32 copies for the BQC/FDC epilogue.
    fp_l32 = nc.dram_tensor("fp_l32", [D, NL], F32, kind="ExternalInput").ap()
    fm_l32 = nc.dram_tensor("fm_l32", [D, NL], F32, kind="ExternalInput").ap()
    b_l32 = nc.dram_tensor("b_l32", [D, NL], F32, kind="ExternalInput").ap()
    out = nc.dram_tensor("out", [D, 8], F32, kind="ExternalOutput").ap()

    with tile.TileContext(nc) as tc:
        with (
            tc.tile_pool(name="consts", bufs=1) as consts,
            tc.tile_pool(name="sstrip", bufs=10) as spool,
            tc.tile_pool(name="ompsum", bufs=2, space="PSUM") as ompool,
            tc.tile_pool(name="upsum", bufs=2, space="PSUM") as upool,
            tc.tile_pool(name="scratch", bufs=1) as scratch,
        ):
            # Small per-core inputs first (they unblock the first matmuls).
            fpl_sb = consts.tile([D, NL], F32R, tag="fpl")
            nc.sync.dma_start(fpl_sb[:], fp_loc[:])
            fplt_sb = consts.tile([128, NL], F32R, tag="fplt")
            nc.sync.dma_start(fplt_sb[:], fpl_t[:])
            fml_sb = consts.tile([D, NL], F32R, tag="fml")
            fmlt_sb = consts.tile([128, NL], F32R, tag="fmlt")
            # fp in halves so the first matmuls start sooner.
            fp_sb = consts.tile([D, N], F32R, tag="fp")
            nc.sync.dma_start(fp_sb[:, : N // 2], fp[:, : N // 2])
            nc.sync.dma_start(fp_sb[:, N // 2 :], fp[:, N // 2 :])
            # fm is not needed until pairing M (~1/3 in); loaded there.
            fm_sb = consts.tile([D, N], F32R, tag="fm")

            acc = consts.tile([D, 64], F32, tag="acc")
            out_sb = consts.tile([D, 8], F32, tag="out")
            nc.vector.memset(out_sb[:], 0.0)

            act_scr = scratch.tile([D, BLK_W], F32, tag="act_scr")
            dve_scr = scratch.tile([D, BLK_W], F32, tag="dve_scr")

            # (S row-block, Omega lhsT [d,i], U lhsT [i,d], rhs / dot tensor)
            pairings = [
                (s_p, fpl_sb, fplt_sb, fp_sb),   # Omega_p: needs only FP
                (s_m, fml_sb, fmlt_sb, fm_sb),   # Omega_m
                (s_u, fpl_sb, fplt_sb, fm_sb),   # Omega_u: lhsT FP, rhs FM
            ]

            def load_fm():
                nc.sync.dma_start(fml_sb[:], fm_loc[:])
                nc.sync.dma_start(fmlt_sb[:], fml_t[:])
                nc.sync.dma_start(fm_sb[:, : N // 2], fm[:, : N // 2])
                nc.sync.dma_start(fm_sb[:, N // 2 :], fm[:, N // 2 :])

            def stream_body(with_fm_load):
                col = 0
                for pi, (s_dram, om_l, u_l, full_sb) in enumerate(pairings):
                    if pi == 1 and with_fm_load:
                        load_fm()
                    for q in range(N_BLOCKS):
                        u_ps = upool.tile([128, BLK_W], F32, tag="u_ps")
                        for m in range(N_STRIPS):
                            s_t = spool.tile([128, BLK_W], F32R, tag="s_t")
                            nc.sync.dma_start(
                                s_t[:],
                                s_dram[
                                    m * 128 : (m + 1) * 128,
                                    q * BLK_W : (q + 1) * BLK_W,
                                ],
                            )
                            om_ps = ompool.tile([128, BLK_W], F32, tag="om_ps")
                            lhsT = om_l[:, m * 128 : (m + 1) * 128]
                            for h in range(2):
                                nc.tensor.matmul(
                                    om_ps[:, h * 512 : (h + 1) * 512],
                                    lhsT,
                                    full_sb[
                                        :,
                                        q * BLK_W + h * 512 : q * BLK_W + (h + 1) * 512,
                                    ],
                                    start=True,
                                    stop=True,
                                )
                            # log(1 + 0.5*g) summed along free dim -> acc col
                            nc.scalar.activation(
                                act_scr[:],
                                om_ps[:],
                                ACTF.Ln,
                                bias=1.0,
                                scale=0.5,
                                accum_out=acc[:, col : col + 1],
                            )
                            col += 1
                            # U[d,j] += F1^T-chunk @ S-tile (accum over strips)
                            ulhsT = u_l[:, m * 128 : (m + 1) * 128]
                            for h in range(2):
                                nc.tensor.matmul(
                                    u_ps[:, h * 512 : (h + 1) * 512],
                                    ulhsT,
                                    s_t[:, h * 512 : (h + 1) * 512],
                                    start=(m == 0),
                                    stop=(m == N_STRIPS - 1),
                                )
                        # sum((U * -0.5) * F2) over the block -> acc col
                        nc.vector.scalar_tensor_tensor(
                            out=dve_scr[:],
                            in0=u_ps[:],
                            scalar=-0.5,
                            in1=full_sb[:, q * BLK_W : (q + 1) * BLK_W],
                            op0=ALU.mult,
                            op1=ALU.mult,
                            accum_out=acc[:, col : col + 1],
                        )
                        col += 1
                assert col == N_MAIN_COLS

            if dyn_repeat is not None:
                load_fm()
                with tc.For_i(0, dyn_repeat, 1):
                    stream_body(with_fm_load=False)
            else:
                for rep in range(repeat):
                    stream_body(with_fm_load=(rep == 0))

            # Epilogue inputs (tiny, stream after the S tiles).
            fpl32_sb = scratch.tile([D, NL], F32, tag="fpl32")
            nc.sync.dma_start(fpl32_sb[:], fp_l32[:])
            fml32_sb = scratch.tile([D, NL], F32, tag="fml32")
            nc.sync.dma_start(fml32_sb[:], fm_l32[:])
            b_sb = scratch.tile([D, NL], F32, tag="b")
            nc.sync.dma_start(b_sb[:], b_l32[:])

            # BQC partials: sum((F_loc - B_loc)^2) per partition. Subtract on
            # GpSimd (otherwise idle), square+accum on DVE.
            bqc_scr = scratch.tile([D, NL], F32, tag="bqc_scr")
            nc.gpsimd.tensor_tensor(
                out=bqc_scr[:], in0=fpl32_sb[:], in1=b_sb[:], op=ALU.subtract
            )
            nc.vector.scalar_tensor_tensor(
                out=bqc_scr[:],
                in0=bqc_scr[:],
                scalar=1.0,
                in1=bqc_scr[:],
                op0=ALU.mult,
                op1=ALU.mult,
                accum_out=acc[:, N_MAIN_COLS : N_MAIN_COLS + 1],
            )
            bqc_scr2 = scratch.tile([D, NL], F32, tag="bqc_scr2")
            nc.gpsimd.tensor_tensor(
                out=bqc_scr2[:], in0=fml32_sb[:], in1=b_sb[:], op=ALU.subtract
            )
            nc.vector.scalar_tensor_tensor(
                out=bqc_scr2[:],
                in0=bqc_scr2[:],
                scalar=1.0,
                in1=bqc_scr2[:],
                op0=ALU.mult,
                op1=ALU.mult,
                accum_out=acc[:, N_MAIN_COLS + 1 : N_MAIN_COLS + 2],
            )

            # FDC rowsum partials over this core's own 512-column slice (the
            # host sums the 8 partials before squaring). ScalarE Copy+accum.
            nc.scalar.activation(
                act_scr[:, :NL],
                fpl32_sb[:],
                ACTF.Copy,
                bias=0.0,
                scale=1.0,
                accum_out=acc[:, N_MAIN_COLS + 2 : N_MAIN_COLS + 3],
            )
            nc.scalar.activation(
                act_scr[:, :NL],
                fml32_sb[:],
                ACTF.Copy,
                bias=0.0,
                scale=1.0,
                accum_out=acc[:, N_MAIN_COLS + 3 : N_MAIN_COLS + 4],
            )

            # Fold the 60 main columns into out col 0.
            nc.vector.tensor_reduce(
                out=out_sb[:, 0:1],
                in_=acc[:, 0:N_MAIN_COLS],
                axis=mybir.AxisListType.X,
                op=ALU.add,
            )
            # bqc_p, bqc_m -> cols 1,2 ; rowsum partials -> cols 3,4
            nc.vector.tensor_copy(
                out_sb[:, 1:5], acc[:, N_MAIN_COLS : N_MAIN_COLS + 4]
            )

            nc.sync.dma_start(out[:], out_sb[:])

    nc.compile()
    return nc


_NC_CACHE = None


def _get_program():
    global _NC_CACHE
    if _NC_CACHE is None:
        _NC_CACHE = build_program()
    return _NC_CACHE


def make_in_maps(SU, SP, SM, FP, FM, B):
    SU = np.ascontiguousarray(np.asarray(SU, np.float32).reshape(N, N))
    SP = np.ascontiguousarray(np.asarray(SP, np.float32).reshape(N, N))
    SM = np.ascontiguousarray(np.asarray(SM, np.float32).reshape(N, N))
    FP = np.ascontiguousarray(np.asarray(FP, np.float32))
    FM = np.ascontiguousarray(np.asarray(FM, np.float32))
    B = np.ascontiguousarray(np.asarray(B, np.float32))

    def packT(X):  # [D, NL] -> [128, NL] with cols m*128+d = X[d, m*128+p]
        # chunk m is X[:, m*128:(m+1)*128].T
        chunks = [
            np.ascontiguousarray(X[:, m * 128 : (m + 1) * 128].T)
            for m in range(N_STRIPS)
        ]
        return np.ascontiguousarray(np.concatenate(chunks, axis=1))

    in_maps = []
    for k in range(N_CORES):
        sl = slice(k * NL, (k + 1) * NL)
        fp_l = np.ascontiguousarray(FP[:, sl])
        fm_l = np.ascontiguousarray(FM[:, sl])
        in_maps.append(
            {
                "s_u": np.ascontiguousarray(SU[sl]),
                "s_p": np.ascontiguousarray(SP[sl]),
                "s_m": np.ascontiguousarray(SM[sl]),
                "fp": FP,
                "fm": FM,
                "fp_loc": fp_l,
                "fm_loc": fm_l,
                "fpl_t": packT(fp_l),
                "fml_t": packT(fm_l),
                "fp_l32": fp_l,
                "fm_l32": fm_l,
                "b_l32": np.ascontiguousarray(B[:, sl]),
            }
        )
    return in_maps


def combine_outs(outs):
    """outs: list of 8 [128, 8] float32 arrays -> scalar loss (float32)."""
    outs = [np.asarray(o, np.float64) for o in outs]
    main = sum(o[:, 0].sum() for o in outs)
    bqc = np.sqrt(sum(o[:, 1].sum() for o in outs)) + np.sqrt(
        sum(o[:, 2].sum() for o in outs)
    )
    rs_fp = sum(o[:, 3] for o in outs)
    rs_fm = sum(o[:, 4] for o in outs)
    fdc = np.square(rs_fp).sum() + np.square(rs_fm).sum()
    return np.float32(main + bqc + fdc)


def kernel(SU, SP, SM, FP, FM, B):
    nc = _get_program()
    in_maps = make_in_maps(SU, SP, SM, FP, FM, B)
    res = run_bass_kernel_spmd(nc, in_maps, list(range(N_CORES)))
    return combine_outs([res.results[k]["out"] for k in range(N_CORES)])


if __name__ == "__main__":
    rng = np.random.default_rng(0)
    ins = {
        "SU": rng.random((N, N, 1), np.float32),
        "SP": rng.random((N, N, 1), np.float32),
        "SM": rng.random((N, N, 1), np.float32),
        "FP": rng.random((D, N), np.float32),
        "FM": rng.random((D, N), np.float32),
        "B": rng.random((D, N), np.float32),
    }
    got = kernel(**ins)
    print("kernel:", got)
